# revision 48
# baseline (speedup 1.0000x reference)
"""Fused single-head attention (QKV projection + softmax(QK^T)V) on 8 trn2 cores.

Problem (hardcoded): x [4, 4096, 768] f32, W_qkv [768, 2304] f32, b_qkv [2304] f32.
  qkv = x @ W_qkv + b_qkv ; q,k,v = split(qkv, 3)
  out = softmax(q k^T / sqrt(768)) v          -> [4, 4096, 768] f32

k-projection elimination: scores_raw = (xWq+bq)(xWk+bk)^T decomposes into
  x (Wq Wk^T) x^T  [computed: u = x @ (M_SCALE*M) on device, scores = u x^T]
  + (x Wq) bk^T    [constant per query row -> cancels in softmax, dropped]
  + bq (x Wk)^T    [per-key scalar d_j = x_j . (Wk bq), host fp32, folded
                    into the Exp activation's per-partition bias]
  + bq bk^T        [constant -> cancels, dropped]
so the device never projects k: the score matmul's key operand is the raw
input x quantized to fp8 on the host (exact round-to-nearest from fp32).
This removes 144 of the 1440 matmuls (~31us of PE time per core).

Sharding: batch (4) x key-halves (2) -> 8 cores. Each core receives only
its own 2048 rows of x (its keys; for the odd core of a pair those are the
batch's rows [2048,4096), sent as that core's rows [0,2048)). It projects
q/k/v for those rows, the pair exchanges q halves with a pairwise
AllGather (placed so both cores hold the full q in unrotated query
order), and each core computes PARTIAL attention sums over its keys:
  outT_partial [768, 4096] = sum_j exp(q k_j^T / sqrt(H)) v_j
  den_partial  [4096]      = sum_j exp(q k_j^T / sqrt(H))
The host combines the partials of each pair: (o0 + o1) / (d0 + d1).
No max-subtraction is needed: scores here are O(1), exp is safe, and both
partials use the same (absent) shift so the combine is exact softmax.

Precision (gate: rel err < 2e-2; measured 1.78e-2, fp16 baseline 3.5e-4):
  - projections run in fp16 (fp8 projection fails the gate: ~2.6e-2).
  - q/k are stored fp8e4 and the score matmuls run DoubleRow (2x PE rate,
    3 matmuls of 256-contraction per 512-query block). Cost ~1.3e-2.
  - PV: the first NF8=8 of 16 key-tiles use fp8 p/v as DoubleRow j-tile
    pairs (6 matmuls per pair instead of 12); the rest fp16. The mean
    component of the v-quantization error is cancelled host-side with a
    rank-1 correction outer(vres, den)/NK, where vres = colsum(v - fp8(v))
    is computed on-device (DVE residual accum + ones-matmul partition sum).

On-chip layout ("transposed flash attention"):
  - qkv computed in head-major layout qT/kT [H, n] via lhsT=W, rhs=xT; v in
    [n, H] via lhsT=xT, rhs=W (no on-chip transposes anywhere).
  - scores computed transposed: sT[j, i] = (kT j-tile).T @ qT -> PSUM,
    exp via ScalarE (scale folded in).
  - denominator: S = sum_jt p_jt accumulated on VectorE (fp32 -> fp16),
    host finishes the partition-dim sum.
  - outT[h, i] += (v j-tile).T @ p accumulated over j in PSUM.
  - PV matmuls run behind QK (software pipeline); PSUM evacuations of
    i-block N are deferred into i-block N+1 (held two j-tiles so the new
    block's exps enqueue on ScalarE ahead of the evacuation copies).
  - phase 1 runs q first and overlaps the AllGather under the k/v passes.
  - DMA: host-interleaved [128, CT|HT, *] layouts make every x r-block / W
    section / output block a single coalesced DMA (each dma_start costs
    ~600ns of issue time on its engine queue).
PSUM budget (8 banks): scores 2 + out accumulators 6.
"""

import math
from contextlib import ExitStack
from functools import lru_cache

import numpy as np

import concourse.bacc as bacc
import concourse.bass as bass
import concourse.tile as tile
from concourse import mybir
from concourse.bass_utils import run_bass_kernel_spmd

B, N, C = 4, 4096, 768
H = 768          # head dim (== C)
H3 = 3 * H
NCORES = 8
NK = N // 2      # keys per core
DT = mybir.dt.float16
F8 = mybir.dt.float8e4
F32 = mybir.dt.float32
SCALE = 1.0 / math.sqrt(H)
M_SCALE = 4.0    # u = x @ (M_SCALE * Wq Wk^T); keeps fp8(u) out of subnormals
USE_DR = True
NF8 = 16         # j-tiles (of JT=16) whose PV runs in fp8 DoubleRow pairs

CT = C // 128    # 6 contraction tiles (c)
HT = H // 128    # 6 head tiles (h)
JT = NK // 128   # 16 key tiles (j) per core
RB = 8           # r-blocks of 512 over the 4096 rows
RBS = N // RB    # 512
KRB = RB // 2    # r-blocks that contain this core's keys (first 4)
IB = 8           # i-blocks of 512 over all 4096 queries
IBS = N // IB    # 512


def build_program():
    nc = bacc.Bacc(
        "TRN2",
        target_bir_lowering=False,
        debug=False,
        enable_asserts=False,
        num_devices=NCORES,
    )
    # x and W are host-interleaved to [128, CT, *] so each r-block / W
    # section is a single coalesced DMA (dma_start costs ~600ns of sync
    # engine time each; the baseline's ~100 phase-1 DMAs serialized there).
    # Each core receives only its own half of the rows (keys == own query
    # half); the other half of q arrives via a pairwise AllGather.
    xT_d = nc.dram_tensor("xT", [128, CT, NK], DT, kind="ExternalInput").ap()
    # host-quantized fp8 copy of this core's x rows: the score matmul's key
    # operand (replaces the k projection entirely)
    x8_d = nc.dram_tensor("x8", [128, CT, NK], F8, kind="ExternalInput").ap()
    # w = [M_SCALE * Wq Wk^T  ||  Wv], interleaved [128, CT, 2H]
    w_d = nc.dram_tensor("w", [128, CT, 2 * H], DT, kind="ExternalInput").ap()
    # per-key exp bias: db[p, jt] = SCALE * x_j . (Wk bq) for key j=jt*128+p
    db_d = nc.dram_tensor("db", [128, JT], F32, kind="ExternalInput").ap()
    outT_d = nc.dram_tensor("outT", [128, HT, N], DT, kind="ExternalOutput").ap()
    # per-partition partial softmax denominators; host sums over axis 1.
    # The v bias AND the fp8-v quantization residual are both rank-1 in
    # (den, h) and are added on the host: out += outer(bv + vres/NK, den).
    den_d = nc.dram_tensor("den", [IB, 128, IBS], F32, kind="ExternalOutput").ap()

    with tile.TileContext(nc) as tc:
        with ExitStack() as ctx:
            persist = ctx.enter_context(tc.tile_pool(name="persist", bufs=1))

            # u/x8 in fp8e4, laid out [128, HT|CT, n] so DoubleRow matmuls can
            # take h-tile pairs via [:, 2g:2g+2, ...] (contraction = 256).
            kT = persist.tile([128, CT, NK], F8, tag="kT", name="kT")
            qT = persist.tile([128, HT, N], F8, tag="qT", name="qT")
            dbs = persist.tile([128, JT], F32, tag="dbs", name="dbs")
            # v for j-tiles < NF8: fp8, stored as j-tile pairs for DoubleRow
            # PV; the rest fp16.
            v8p = [persist.tile([128, 2, H], F8, tag=f"v8p{t}", name=f"v8p{t}")
                   for t in range(NF8 // 2)]
            vv = {t: persist.tile([128, H], DT, tag=f"v{t}", name=f"v{t}")
                  for t in range(NF8, JT)}
            dramp = ctx.enter_context(
                tc.tile_pool(name="dramp", bufs=1, space="DRAM"))
            q_inb = dramp.tile([KRB, 128, HT, RBS], F8, name="q_inb")
            q_outb = dramp.tile([KRB, 2, 128, HT, RBS], F8, name="q_outb")

            # ---- Phase 1: QKV projection ----
            with tc.tile_pool(name="wpool", bufs=1) as wpool, \
                 tc.tile_pool(name="xpool", bufs=4) as xpool, \
                 tc.tile_pool(name="pj", bufs=4, space="PSUM") as pj, \
                 tc.tile_pool(name="pv", bufs=2, space="PSUM") as pv:

                ws = wpool.tile([128, CT, 2 * H], DT, tag="ws", name="ws")
                # own-half u staging, r-block-major so each r-block is one
                # contiguous chunk for its pipelined pairwise AllGather
                qstage = wpool.tile([128, KRB, HT, RBS], F8, tag="qstage",
                                    name="qstage")

                def load_xt(rb):
                    r0 = rb * RBS
                    t = xpool.tile([128, CT, RBS], DT, tag="xt", name=f"xt{rb}")
                    nc.sync.dma_start(out=t, in_=xT_d[:, :, r0:r0 + RBS])
                    return t

                # DMA issue order = need order: u runs first (its result
                # feeds the cross-core exchange), so M's columns and the
                # first x block are granular (per-ct) so the first matmuls
                # start as soon as their chunk lands; the rest one coalesced
                # DMA per section (each dma_start costs ~600ns of
                # sync-engine issue time).
                # Only the 4 DMAs the first projection chain needs are issued
                # before it: semaphore waits coarsen to "all prior DMAs on
                # this queue", so anything issued here would delay the first
                # matmul. The rest are issued between the rb0 chains below.
                xts = [None] * KRB
                xt0 = xpool.tile([128, CT, RBS], DT, tag="xt", name="xt0")
                hc = CT // 2
                nc.sync.dma_start(out=ws[:, 0:hc, 0:H], in_=w_d[:, 0:hc, 0:H])
                nc.sync.dma_start(out=xt0[:, 0:hc, :],
                                  in_=xT_d[:, 0:hc, 0:RBS])
                nc.sync.dma_start(out=ws[:, hc:CT, 0:H],
                                  in_=w_d[:, hc:CT, 0:H])
                nc.sync.dma_start(out=xt0[:, hc:CT, :],
                                  in_=xT_d[:, hc:CT, 0:RBS])
                xts[0] = xt0

                def late_dmas(rb, ht):
                    # issued between the rb0/rb1 u chains, after the first
                    # matmuls, so the first chain's (coarse) DMA wait covers
                    # as few bytes as possible
                    if rb == 0:
                        if ht == 0:
                            nc.sync.dma_start(out=dbs, in_=db_d)
                            xts[1] = load_xt(1)
                        elif ht == 1:
                            # score matmul's key operand: host-quantized fp8
                            # x, straight into its persistent SBUF home
                            nc.scalar.dma_start(out=kT, in_=x8_d)
                        elif ht == 2:
                            xts[2] = load_xt(2)
                        elif ht == 3:
                            nc.sync.dma_start(out=ws[:, :, H:2 * H],
                                              in_=w_d[:, :, H:2 * H])
                        elif ht == 4:
                            xts[3] = load_xt(3)

                # PE warm-up: ~3.4us of junk matmuls (no DMA deps) so the
                # HAM clock-gate reaches full rate while the first x/W DMAs
                # are still in flight.
                warm_l = xpool.tile([128, 128], DT, tag="warml", name="warml")
                warm_r = xpool.tile([128, 512], DT, tag="warmr", name="warmr")
                nc.gpsimd.memset(warm_l, 0.0)
                nc.gpsimd.memset(warm_r, 0.0)
                # single accumulation chain: no per-MM semaphore round-trips.
                # Sized so the PE stays busy until the first chain's DMAs
                # land (~15us): an idle gap resets the p-state ramp.
                NWARM = 16
                wp = pj.tile([128, RBS], F32, tag="pj", name="warm")
                for i in range(NWARM):
                    nc.tensor.matmul(wp, warm_l, warm_r,
                                     start=(i == 0), stop=(i == NWARM - 1))

                def proj_u(rb):
                    xt = xts[rb]
                    for ht in range(HT):
                        ps = pj.tile([128, RBS], F32, tag="pj")
                        for ct in range(CT):
                            nc.tensor.matmul(
                                ps,
                                ws[:, ct, ht * 128:(ht + 1) * 128],
                                xt[:, ct, :],
                                start=(ct == 0), stop=(ct == CT - 1),
                            )
                        nc.scalar.activation(
                            out=qstage[:, rb, ht, :],
                            in_=ps,
                            func=mybir.ActivationFunctionType.Identity,
                        )
                        late_dmas(rb, ht)

                # pass 1: u = x @ (M_SCALE * Wq Wk^T), one pipelined pairwise
                # AllGather per r-block so the exchange streams behind the
                # projection instead of serializing after it. Both cores of a
                # pair end up with the full u in unrotated query order (the
                # host therefore does not rotate the odd cores' outputs back).
                def gather_u(rb):
                    r0 = rb * RBS
                    nc.gpsimd.dma_start(out=q_inb[rb], in_=qstage[:, rb])
                    nc.gpsimd.collective_compute(
                        "AllGather",
                        mybir.AluOpType.bypass,
                        replica_groups=[[2 * i, 2 * i + 1]
                                        for i in range(NCORES // 2)],
                        ins=[q_inb[rb].opt()],
                        outs=[q_outb[rb].opt()],
                    )
                    nc.gpsimd.dma_start(out=qT[:, :, r0:r0 + RBS],
                                        in_=q_outb[rb, 0])
                    nc.gpsimd.dma_start(out=qT[:, :, NK + r0:NK + r0 + RBS],
                                        in_=q_outb[rb, 1])

                def proj_v(rb):
                    xt = xts[rb]
                    if True:
                        for j in range(RBS // 128):
                            jt = rb * (RBS // 128) + j
                            ps = pv.tile([128, H], F32, tag="pv")
                            for ct in range(CT):
                                xs = xt[:, ct, j * 128:(j + 1) * 128]
                                nc.tensor.matmul(
                                    ps[:, 0:512], xs, ws[:, ct, H: H + 512],
                                    start=(ct == 0), stop=(ct == CT - 1))
                                nc.tensor.matmul(
                                    ps[:, 512:H], xs, ws[:, ct, H + 512: 2 * H],
                                    start=(ct == 0), stop=(ct == CT - 1))
                            # v is stored WITHOUT bv (host adds outer(bv,den))
                            # so fp8 tiles quantize straight from PSUM; the
                            # quantization residual's column-sum is replicated
                            # exactly on the host (it knows x, Wv and the
                            # rounding) for the rank-1 correction.
                            if jt < NF8:
                                nc.scalar.activation(
                                    out=v8p[jt // 2][:, jt % 2, :], in_=ps,
                                    func=mybir.ActivationFunctionType.Copy)
                            else:
                                nc.vector.tensor_copy(out=vv[jt], in_=ps)

                # u/v interleaved per r-block: each r-block's u feeds its
                # pipelined AllGather, and the v work covers the collectives'
                # latency (the first one pays ~20us of rendezvous setup)
                for rb in range(KRB):
                    proj_u(rb)
                    gather_u(rb)
                    proj_v(rb)

            # ---- Phase 2: attention (partial sums over this core's keys) ----
            with tc.tile_pool(name="ppool", bufs=1) as ppool, \
                 tc.tile_pool(name="opool", bufs=2) as opool, \
                 tc.tile_pool(name="spool", bufs=2) as spool, \
                 tc.tile_pool(name="tpool", bufs=3) as tpool, \
                 tc.tile_pool(name="ps_s", bufs=2, space="PSUM") as ps_s, \
                 tc.tile_pool(name="ps_o", bufs=3, space="PSUM") as ps_o:
                p8_t = [ppool.tile([128, 2, IBS], F8, tag=f"p8{t}",
                                   name=f"p8{t}") for t in range(NF8 // 2)]
                p_t = {t: ppool.tile([128, IBS], DT, tag=f"p{t}", name=f"p{t}")
                       for t in range(NF8, JT)}

                def p_ap(jt):
                    if jt < NF8:
                        return p8_t[jt // 2][:, jt % 2, :]
                    return p_t[jt]

                pending = []   # deferred work, flushed between PE groups

                def flush():
                    while pending:
                        pending.pop(0)()

                def emit_pv8(og, pr):
                    # one DoubleRow group covers the j-tile pair (2pr, 2pr+1)
                    def go():
                        for ht in range(HT):
                            nc.tensor.matmul(
                                og[ht],
                                v8p[pr][:, :, ht * 128:(ht + 1) * 128],
                                p8_t[pr],
                                start=(pr == 0),
                                stop=(NF8 == JT and pr == NF8 // 2 - 1),
                                perf_mode=mybir.MatmulPerfMode.DoubleRow,
                            )
                    pending.append(go)

                def emit_pv(og, jt, i0):
                    def go():
                        for ht in range(HT):
                            nc.tensor.matmul(
                                og[ht],
                                vv[jt][:, ht * 128:(ht + 1) * 128],
                                p_t[jt],
                                start=(jt == 0 and NF8 == 0),
                                stop=(jt == JT - 1),
                            )
                    pending.append(go)

                def emit_den_and_evac(og2, Sf, ib, i0):
                    def go():
                        nc.sync.dma_start(out=den_d[ib], in_=Sf)
                        ot = opool.tile([128, HT, IBS], DT, tag="ot",
                                        name=f"ot{i0}")
                        for g in range(HT // 2):
                            # one double-width copy evacuates a 2-bank tile
                            if g % 2 == 0:
                                nc.vector.tensor_copy(
                                    out=ot[:, 2 * g:2 * g + 2, :], in_=og2[g])
                            else:
                                nc.scalar.activation(
                                    out=ot[:, 2 * g:2 * g + 2, :], in_=og2[g],
                                    func=mybir.ActivationFunctionType.Copy)
                        nc.scalar.dma_start(out=outT_d[:, :, i0:i0 + IBS],
                                            in_=ot)
                    pending.append(go)

                for ib in range(IB):
                    i0 = ib * IBS
                    og2 = [ps_o.tile([128, 2 * IBS], F32, tag="o",
                                     name=f"o{ib}_{g}")
                           for g in range(HT // 2)]
                    og = [og2[g // 2][:, (g % 2) * IBS:(g % 2 + 1) * IBS]
                          for g in range(HT)]
                    Sf = spool.tile([128, IBS], F32, tag="Sf", name=f"Sf{ib}")
                    for jt in range(JT):
                        sps = ps_s.tile([128, IBS], F32, tag="s")
                        # hold the previous i-block's PV/evacuation flush for
                        # two j-tiles so this block's first exps enqueue on
                        # ScalarE ahead of the evacuation copies
                        hold_flush = (jt < 2 and ib > 0)
                        if USE_DR:
                            for g in range(HT // 2):
                                nc.tensor.matmul(
                                    sps,
                                    kT[:, 2 * g:2 * g + 2,
                                       jt * 128:(jt + 1) * 128],
                                    qT[:, 2 * g:2 * g + 2, i0:i0 + IBS],
                                    start=(g == 0), stop=(g == HT // 2 - 1),
                                    perf_mode=mybir.MatmulPerfMode.DoubleRow,
                                )
                        else:
                            for ht in range(HT):
                                nc.tensor.matmul(
                                    sps,
                                    kT[:, ht, jt * 128:(jt + 1) * 128],
                                    qT[:, ht, i0:i0 + IBS],
                                    start=(ht == 0), stop=(ht == HT - 1),
                                )
                        if not hold_flush:
                            flush()
                        # p-1 shift: store fp8(exp(s)-1) -- ~3.6x finer fp8
                        # quantization since exp args are ~N(0,0.33). The
                        # host adds back the exact +colsum(v8) rank-1 term
                        # and +NK to den. The subtract runs on the otherwise
                        # idle GpSimd engine.
                        pt = tpool.tile([128, IBS], DT, tag="pt")
                        nc.scalar.activation(
                            out=pt, in_=sps,
                            func=mybir.ActivationFunctionType.Exp,
                            scale=SCALE / M_SCALE,
                            bias=dbs[:, jt:jt + 1],
                        )
                        nc.gpsimd.tensor_scalar_sub(p_ap(jt), pt, 1.0)
                        if jt == 0:
                            nc.vector.tensor_copy(out=Sf, in_=p_ap(jt))
                        else:
                            nc.vector.tensor_add(Sf, Sf, p_ap(jt))
                        # pair 0 is deferred one extra j-tile so the previous
                        # i-block's PSUM evacuation (DVE/ScalarE) finishes
                        # before its banks are re-accumulated
                        if jt == 2 and NF8 >= 2:
                            emit_pv8(og, 0)
                        if jt < NF8:
                            if (jt % 2 == 1 and jt > 1
                                    and not (ib == IB - 1 and jt == JT - 1)):
                                emit_pv8(og, jt // 2)
                        elif not (ib == IB - 1 and jt == JT - 1):
                            emit_pv(og, jt, i0)
                    if ib < IB - 1:
                        emit_den_and_evac(og2, Sf, ib, i0)
                    else:
                        # eager epilogue: interleave the final j-tile pair's
                        # PV matmuls with per-SINGLE-h-tile evacuation and
                        # DMA so the output drains as early as possible.
                        def epilogue(og=og, og2=og2, Sf=Sf, ib=ib, i0=i0):
                            nc.sync.dma_start(out=den_d[ib], in_=Sf)
                            ot = opool.tile([128, HT, IBS], DT, tag="ot",
                                            name=f"ot{i0}")
                            for ht in range(HT):
                                if NF8 == JT:
                                    nc.tensor.matmul(
                                        og[ht],
                                        v8p[JT // 2 - 1][:, :,
                                                         ht * 128:(ht + 1) * 128],
                                        p8_t[JT // 2 - 1],
                                        start=False, stop=True,
                                        perf_mode=mybir.MatmulPerfMode.DoubleRow,
                                    )
                                else:
                                    nc.tensor.matmul(
                                        og[ht],
                                        vv[JT - 1][:, ht * 128:(ht + 1) * 128],
                                        p_t[JT - 1],
                                        start=False, stop=True,
                                    )
                                if ht % 2 == 1:
                                    g = ht // 2
                                    if g % 2 == 0:
                                        nc.vector.tensor_copy(
                                            out=ot[:, ht - 1:ht + 1, :],
                                            in_=og2[g])
                                    else:
                                        nc.scalar.activation(
                                            out=ot[:, ht - 1:ht + 1, :],
                                            in_=og2[g],
                                            func=mybir.ActivationFunctionType.Copy)
                                    dma = (nc.sync.dma_start if ht % 4 == 1
                                           else nc.scalar.dma_start)
                                    dma(out=outT_d[:, ht - 1:ht + 1,
                                                   i0:i0 + IBS],
                                        in_=ot[:, ht - 1:ht + 1, :])
                        pending.append(epilogue)
                flush()
    nc.compile()
    return nc


@lru_cache(maxsize=1)
def _cached_program():
    return build_program()


def _prep_in_maps(x, W_qkv, b_qkv):
    import ml_dtypes

    x = np.asarray(x, dtype=np.float32)
    W_qkv = np.asarray(W_qkv, dtype=np.float32)
    b_qkv = np.asarray(b_qkv, dtype=np.float32)
    Wq, Wk, Wv = W_qkv[:, 0:H], W_qkv[:, H:2 * H], W_qkv[:, 2 * H:3 * H]
    bq = b_qkv[0:H]
    # scores_raw = x (Wq Wk^T) x^T + per-key bias d; per-query terms cancel
    M = (M_SCALE * (Wq @ Wk.T)).astype(np.float32)           # [C, C]
    w3 = Wk @ bq                                             # [C]
    # interleave [C, 2H] -> [128, CT, 2H] so W sections are single DMAs
    w16 = np.ascontiguousarray(
        np.concatenate([M, Wv], axis=1)
        .astype(np.float16).reshape(CT, 128, 2 * H).transpose(1, 0, 2))
    bv = b_qkv[2 * H:3 * H].astype(np.float32)
    Wv16 = Wv.astype(np.float16).astype(np.float32)

    in_maps = []
    hcorrs = []
    for core in range(NCORES):
        b, kh = core // 2, core % 2
        # this core's rows: keys == own query half
        xb = x[b][kh * NK:(kh + 1) * NK]   # [NK, C] f32
        xTf = xb.T.reshape(CT, 128, NK).transpose(1, 0, 2)
        xT = np.ascontiguousarray(xTf.astype(np.float16))
        x8 = np.ascontiguousarray(xTf.astype(ml_dtypes.float8_e4m3fn))
        db = np.ascontiguousarray(
            (SCALE * (xb @ w3)).astype(np.float32).reshape(JT, 128).T)
        in_maps.append({"xT": xT, "x8": x8, "w": w16, "db": db})
        # replicate the device's v (fp16 x, fp16 Wv, fp32 accum, no bias) and
        # its fp8 quantization; vres = column-sum of the residual over the
        # fp8-stored keys. Host adds out += outer(bv + vres/NK, den) and the
        # p-1 shift's exact +colsum(v8) term.
        vdev = xb.astype(np.float16).astype(np.float32) @ Wv16
        v8 = vdev[0:NF8 * 128].astype(ml_dtypes.float8_e4m3fn).astype(np.float32)
        vres = (vdev[0:NF8 * 128] - v8).sum(axis=0)
        vsum = v8.sum(axis=0) + vdev[NF8 * 128:].astype(
            np.float16).astype(np.float32).sum(axis=0)
        hcorrs.append((bv + vres / NK, vsum))
    return in_maps, hcorrs


def _unT(o):
    # [128, HT, N] fp16 -> [H, N] fp32
    return o.astype(np.float32).transpose(1, 0, 2).reshape(H, N)


def _combine(results, hcorrs):
    out = np.empty((B, N, C), dtype=np.float32)
    for b in range(B):
        r0, r1 = results[2 * b], results[2 * b + 1]
        o0 = _unT(r0["outT"])                    # [H, N]
        d0 = r0["den"].astype(np.float32).sum(axis=1).reshape(N) + NK
        o1 = _unT(r1["outT"])
        d1 = r1["den"].astype(np.float32).sum(axis=1).reshape(N) + NK
        # rank-1 corrections: the p-1 shift's +colsum(v8) term, the v bias,
        # and the fp8-v quantization residual mean. (Both cores' outputs are
        # already in unrotated query order thanks to the AllGather placement.)
        hc0, vs0 = hcorrs[2 * b]
        hc1, vs1 = hcorrs[2 * b + 1]
        o0 = o0 + vs0[:, None] + np.outer(hc0, d0)
        o1 = o1 + vs1[:, None] + np.outer(hc1, d1)
        out[b] = ((o0 + o1) / (d0 + d1)).T
    return out


def kernel(x, W_qkv, b_qkv):
    nc = _cached_program()
    in_maps, hcorrs = _prep_in_maps(x, W_qkv, b_qkv)
    res = run_bass_kernel_spmd(nc, in_maps, core_ids=list(range(NCORES)))
    return _combine(res.results, hcorrs)



# revision 51
# speedup vs baseline: 4.0951x; 4.0951x over previous
"""Fused single-head attention (QKV projection + softmax(QK^T)V) on 8 trn2 cores.

Problem (hardcoded): x [4, 4096, 768] f32, W_qkv [768, 2304] f32, b_qkv [2304] f32.
  qkv = x @ W_qkv + b_qkv ; q,k,v = split(qkv, 3)
  out = softmax(q k^T / sqrt(768)) v          -> [4, 4096, 768] f32

k-projection elimination: scores_raw = (xWq+bq)(xWk+bk)^T decomposes into
  x (Wq Wk^T) x^T  [computed: u = x @ (M_SCALE*M) on device, scores = u x^T]
  + (x Wq) bk^T    [constant per query row -> cancels in softmax, dropped]
  + bq (x Wk)^T    [per-key scalar d_j = x_j . (Wk bq), host fp32, folded
                    into the Exp activation's per-partition bias]
  + bq bk^T        [constant -> cancels, dropped]
so the device never projects k: the score matmul's key operand is the raw
input x quantized to fp8 on the host (exact round-to-nearest from fp32).
This removes 144 of the 1440 matmuls (~31us of PE time per core).

Sharding: batch (4) x key-halves (2) -> 8 cores. Each core receives only
its own 2048 rows of x (its keys; for the odd core of a pair those are the
batch's rows [2048,4096), sent as that core's rows [0,2048)). It projects
q/k/v for those rows, the pair exchanges q halves with a pairwise
AllGather (placed so both cores hold the full q in unrotated query
order), and each core computes PARTIAL attention sums over its keys:
  outT_partial [768, 4096] = sum_j exp(q k_j^T / sqrt(H)) v_j
  den_partial  [4096]      = sum_j exp(q k_j^T / sqrt(H))
The host combines the partials of each pair: (o0 + o1) / (d0 + d1).
No max-subtraction is needed: scores here are O(1), exp is safe, and both
partials use the same (absent) shift so the combine is exact softmax.

Precision (gate: rel err < 2e-2; measured 1.78e-2, fp16 baseline 3.5e-4):
  - projections run in fp16 (fp8 projection fails the gate: ~2.6e-2).
  - q/k are stored fp8e4 and the score matmuls run DoubleRow (2x PE rate,
    3 matmuls of 256-contraction per 512-query block). Cost ~1.3e-2.
  - PV: the first NF8=8 of 16 key-tiles use fp8 p/v as DoubleRow j-tile
    pairs (6 matmuls per pair instead of 12); the rest fp16. The mean
    component of the v-quantization error is cancelled host-side with a
    rank-1 correction outer(vres, den)/NK, where vres = colsum(v - fp8(v))
    is computed on-device (DVE residual accum + ones-matmul partition sum).

On-chip layout ("transposed flash attention"):
  - qkv computed in head-major layout qT/kT [H, n] via lhsT=W, rhs=xT; v in
    [n, H] via lhsT=xT, rhs=W (no on-chip transposes anywhere).
  - scores computed transposed: sT[j, i] = (kT j-tile).T @ qT -> PSUM,
    exp via ScalarE (scale folded in).
  - denominator: S = sum_jt p_jt accumulated on VectorE (fp32 -> fp16),
    host finishes the partition-dim sum.
  - outT[h, i] += (v j-tile).T @ p accumulated over j in PSUM.
  - PV matmuls run behind QK (software pipeline); PSUM evacuations of
    i-block N are deferred into i-block N+1 (held two j-tiles so the new
    block's exps enqueue on ScalarE ahead of the evacuation copies).
  - phase 1 runs q first and overlaps the AllGather under the k/v passes.
  - DMA: host-interleaved [128, CT|HT, *] layouts make every x r-block / W
    section / output block a single coalesced DMA (each dma_start costs
    ~600ns of issue time on its engine queue).
PSUM budget (8 banks): scores 2 + out accumulators 6.
"""

import math
from contextlib import ExitStack
from functools import lru_cache

import numpy as np

import concourse.bacc as bacc
import concourse.bass as bass
import concourse.tile as tile
from concourse import mybir
from concourse.bass_utils import run_bass_kernel_spmd

B, N, C = 4, 4096, 768
H = 768          # head dim (== C)
H3 = 3 * H
NCORES = 8
NK = N // 2      # keys per core
DT = mybir.dt.float16
F8 = mybir.dt.float8e4
F32 = mybir.dt.float32
SCALE = 1.0 / math.sqrt(H)
M_SCALE = 4.0    # u = x @ (M_SCALE * Wq Wk^T); keeps fp8(u) out of subnormals
USE_DR = True
NF8 = 16         # j-tiles (of JT=16) whose PV runs in fp8 DoubleRow pairs

CT = C // 128    # 6 contraction tiles (c)
HT = H // 128    # 6 head tiles (h)
JT = NK // 128   # 16 key tiles (j) per core
RB = 8           # r-blocks of 512 over the 4096 rows
RBS = N // RB    # 512
KRB = RB // 2    # r-blocks that contain this core's keys (first 4)
IB = 8           # i-blocks of 512 over all 4096 queries
IBS = N // IB    # 512


def build_program():
    nc = bacc.Bacc(
        "TRN2",
        target_bir_lowering=False,
        debug=False,
        enable_asserts=False,
        num_devices=NCORES,
    )
    # x and W are host-interleaved to [128, CT, *] so each r-block / W
    # section is a single coalesced DMA (dma_start costs ~600ns of sync
    # engine time each; the baseline's ~100 phase-1 DMAs serialized there).
    # Each core receives only its own half of the rows (keys == own query
    # half); the other half of q arrives via a pairwise AllGather.
    xT_d = nc.dram_tensor("xT", [128, CT, NK], DT, kind="ExternalInput").ap()
    # host-quantized fp8 copy of this core's x rows: the score matmul's key
    # operand (replaces the k projection entirely)
    x8_d = nc.dram_tensor("x8", [128, CT, NK], F8, kind="ExternalInput").ap()
    # w = [M_SCALE * Wq Wk^T  ||  Wv], interleaved [128, CT, 2H]
    w_d = nc.dram_tensor("w", [128, CT, 2 * H], DT, kind="ExternalInput").ap()
    # per-key exp bias: db[p, jt] = SCALE * x_j . (Wk bq) for key j=jt*128+p
    db_d = nc.dram_tensor("db", [128, JT], F32, kind="ExternalInput").ap()
    outT_d = nc.dram_tensor("outT", [128, HT, N], DT, kind="ExternalOutput").ap()
    # per-partition partial softmax denominators; host sums over axis 1.
    # The v bias AND the fp8-v quantization residual are both rank-1 in
    # (den, h) and are added on the host: out += outer(bv + vres/NK, den).
    den_d = nc.dram_tensor("den", [IB, 128, IBS], F32, kind="ExternalOutput").ap()

    with tile.TileContext(nc) as tc:
        with ExitStack() as ctx:
            persist = ctx.enter_context(tc.tile_pool(name="persist", bufs=1))

            # u/x8 in fp8e4, laid out [128, HT|CT, n] so DoubleRow matmuls can
            # take h-tile pairs via [:, 2g:2g+2, ...] (contraction = 256).
            kT = persist.tile([128, CT, NK], F8, tag="kT", name="kT")
            qT = persist.tile([128, HT, N], F8, tag="qT", name="qT")
            dbs = persist.tile([128, JT], F32, tag="dbs", name="dbs")
            neg1 = persist.tile([128, 1], F32, tag="neg1", name="neg1")
            nc.gpsimd.memset(neg1, -1.0)
            # v for j-tiles < NF8: fp8, stored as j-tile pairs for DoubleRow
            # PV; the rest fp16.
            v8p = [persist.tile([128, 2, H], F8, tag=f"v8p{t}", name=f"v8p{t}")
                   for t in range(NF8 // 2)]
            vv = {t: persist.tile([128, H], DT, tag=f"v{t}", name=f"v{t}")
                  for t in range(NF8, JT)}
            dramp = ctx.enter_context(
                tc.tile_pool(name="dramp", bufs=1, space="DRAM"))
            q_inb = dramp.tile([KRB, 128, HT, RBS], F8, name="q_inb")
            q_outb = dramp.tile([KRB, 2, 128, HT, RBS], F8, name="q_outb")

            # ---- Phase 1: QKV projection ----
            with tc.tile_pool(name="wpool", bufs=1) as wpool, \
                 tc.tile_pool(name="xpool", bufs=4) as xpool, \
                 tc.tile_pool(name="pj", bufs=4, space="PSUM") as pj, \
                 tc.tile_pool(name="pv", bufs=2, space="PSUM") as pv:

                ws = wpool.tile([128, CT, 2 * H], DT, tag="ws", name="ws")
                # own-half u staging, r-block-major so each r-block is one
                # contiguous chunk for its pipelined pairwise AllGather
                qstage = wpool.tile([128, KRB, HT, RBS], F8, tag="qstage",
                                    name="qstage")

                def load_xt(rb):
                    r0 = rb * RBS
                    t = xpool.tile([128, CT, RBS], DT, tag="xt", name=f"xt{rb}")
                    nc.sync.dma_start(out=t, in_=xT_d[:, :, r0:r0 + RBS])
                    return t

                # DMA issue order = need order: u runs first (its result
                # feeds the cross-core exchange), so M's columns and the
                # first x block are granular (per-ct) so the first matmuls
                # start as soon as their chunk lands; the rest one coalesced
                # DMA per section (each dma_start costs ~600ns of
                # sync-engine issue time).
                # Only the 4 DMAs the first projection chain needs are issued
                # before it: semaphore waits coarsen to "all prior DMAs on
                # this queue", so anything issued here would delay the first
                # matmul. The rest are issued between the rb0 chains below.
                xts = [None] * KRB
                xt0 = xpool.tile([128, CT, RBS], DT, tag="xt", name="xt0")
                hc = CT // 2
                nc.sync.dma_start(out=ws[:, 0:hc, 0:H], in_=w_d[:, 0:hc, 0:H])
                nc.sync.dma_start(out=xt0[:, 0:hc, :],
                                  in_=xT_d[:, 0:hc, 0:RBS])
                nc.sync.dma_start(out=ws[:, hc:CT, 0:H],
                                  in_=w_d[:, hc:CT, 0:H])
                nc.sync.dma_start(out=xt0[:, hc:CT, :],
                                  in_=xT_d[:, hc:CT, 0:RBS])
                xts[0] = xt0

                def late_dmas(rb, ht):
                    # issued between the rb0/rb1 u chains, after the first
                    # matmuls, so the first chain's (coarse) DMA wait covers
                    # as few bytes as possible
                    if rb == 0:
                        if ht == 0:
                            nc.sync.dma_start(out=dbs, in_=db_d)
                            xts[1] = load_xt(1)
                        elif ht == 1:
                            # score matmul's key operand: host-quantized fp8
                            # x, straight into its persistent SBUF home
                            nc.scalar.dma_start(out=kT, in_=x8_d)
                        elif ht == 2:
                            xts[2] = load_xt(2)
                        elif ht == 3:
                            nc.sync.dma_start(out=ws[:, :, H:2 * H],
                                              in_=w_d[:, :, H:2 * H])
                        elif ht == 4:
                            xts[3] = load_xt(3)

                # PE warm-up: ~3.4us of junk matmuls (no DMA deps) so the
                # HAM clock-gate reaches full rate while the first x/W DMAs
                # are still in flight.
                warm_l = xpool.tile([128, 128], DT, tag="warml", name="warml")
                warm_r = xpool.tile([128, 512], DT, tag="warmr", name="warmr")
                nc.gpsimd.memset(warm_l, 0.0)
                nc.gpsimd.memset(warm_r, 0.0)
                # single accumulation chain: no per-MM semaphore round-trips.
                # Sized so the PE stays busy until the first chain's DMAs
                # land (~15us): an idle gap resets the p-state ramp.
                NWARM = 16
                wp = pj.tile([128, RBS], F32, tag="pj", name="warm")
                for i in range(NWARM):
                    nc.tensor.matmul(wp, warm_l, warm_r,
                                     start=(i == 0), stop=(i == NWARM - 1))

                def proj_u(rb):
                    xt = xts[rb]
                    for ht in range(HT):
                        ps = pj.tile([128, RBS], F32, tag="pj")
                        for ct in range(CT):
                            nc.tensor.matmul(
                                ps,
                                ws[:, ct, ht * 128:(ht + 1) * 128],
                                xt[:, ct, :],
                                start=(ct == 0), stop=(ct == CT - 1),
                            )
                        nc.scalar.activation(
                            out=qstage[:, rb, ht, :],
                            in_=ps,
                            func=mybir.ActivationFunctionType.Identity,
                        )
                        late_dmas(rb, ht)

                # pass 1: u = x @ (M_SCALE * Wq Wk^T), one pipelined pairwise
                # AllGather per r-block so the exchange streams behind the
                # projection instead of serializing after it. Both cores of a
                # pair end up with the full u in unrotated query order (the
                # host therefore does not rotate the odd cores' outputs back).
                def gather_u(rb):
                    r0 = rb * RBS
                    nc.gpsimd.dma_start(out=q_inb[rb], in_=qstage[:, rb])
                    nc.gpsimd.collective_compute(
                        "AllGather",
                        mybir.AluOpType.bypass,
                        replica_groups=[[2 * i, 2 * i + 1]
                                        for i in range(NCORES // 2)],
                        ins=[q_inb[rb].opt()],
                        outs=[q_outb[rb].opt()],
                    )
                    nc.gpsimd.dma_start(out=qT[:, :, r0:r0 + RBS],
                                        in_=q_outb[rb, 0])
                    nc.gpsimd.dma_start(out=qT[:, :, NK + r0:NK + r0 + RBS],
                                        in_=q_outb[rb, 1])

                def proj_v(rb):
                    xt = xts[rb]
                    if True:
                        for j in range(RBS // 128):
                            jt = rb * (RBS // 128) + j
                            ps = pv.tile([128, H], F32, tag="pv")
                            for ct in range(CT):
                                xs = xt[:, ct, j * 128:(j + 1) * 128]
                                nc.tensor.matmul(
                                    ps[:, 0:512], xs, ws[:, ct, H: H + 512],
                                    start=(ct == 0), stop=(ct == CT - 1))
                                nc.tensor.matmul(
                                    ps[:, 512:H], xs, ws[:, ct, H + 512: 2 * H],
                                    start=(ct == 0), stop=(ct == CT - 1))
                            # v is stored WITHOUT bv (host adds outer(bv,den))
                            # so fp8 tiles quantize straight from PSUM; the
                            # quantization residual's column-sum is replicated
                            # exactly on the host (it knows x, Wv and the
                            # rounding) for the rank-1 correction.
                            if jt < NF8:
                                nc.scalar.activation(
                                    out=v8p[jt // 2][:, jt % 2, :], in_=ps,
                                    func=mybir.ActivationFunctionType.Copy)
                            else:
                                nc.vector.tensor_copy(out=vv[jt], in_=ps)

                # u/v interleaved per r-block: each r-block's u feeds its
                # pipelined AllGather, and the v work covers the collectives'
                # latency (the first one pays ~20us of rendezvous setup)
                for rb in range(KRB):
                    proj_u(rb)
                    gather_u(rb)
                    proj_v(rb)

            # ---- Phase 2: attention (partial sums over this core's keys) ----
            with tc.tile_pool(name="ppool", bufs=1) as ppool, \
                 tc.tile_pool(name="opool", bufs=2) as opool, \
                 tc.tile_pool(name="spool", bufs=2) as spool, \
                 tc.tile_pool(name="tpool", bufs=3) as tpool, \
                 tc.tile_pool(name="ps_s", bufs=2, space="PSUM") as ps_s, \
                 tc.tile_pool(name="ps_o", bufs=3, space="PSUM") as ps_o:
                p8_t = [ppool.tile([128, 2, IBS], F8, tag=f"p8{t}",
                                   name=f"p8{t}") for t in range(NF8 // 2)]
                p_t = {t: ppool.tile([128, IBS], DT, tag=f"p{t}", name=f"p{t}")
                       for t in range(NF8, JT)}

                def p_ap(jt):
                    if jt < NF8:
                        return p8_t[jt // 2][:, jt % 2, :]
                    return p_t[jt]

                pending = []   # deferred work, flushed between PE groups

                def flush():
                    while pending:
                        pending.pop(0)()

                def emit_pv8(og, pr):
                    # one DoubleRow group covers the j-tile pair (2pr, 2pr+1)
                    def go():
                        for ht in range(HT):
                            nc.tensor.matmul(
                                og[ht],
                                v8p[pr][:, :, ht * 128:(ht + 1) * 128],
                                p8_t[pr],
                                start=(pr == 0),
                                stop=(NF8 == JT and pr == NF8 // 2 - 1),
                                perf_mode=mybir.MatmulPerfMode.DoubleRow,
                            )
                    pending.append(go)

                def emit_pv(og, jt, i0):
                    def go():
                        for ht in range(HT):
                            nc.tensor.matmul(
                                og[ht],
                                vv[jt][:, ht * 128:(ht + 1) * 128],
                                p_t[jt],
                                start=(jt == 0 and NF8 == 0),
                                stop=(jt == JT - 1),
                            )
                    pending.append(go)

                def emit_den_and_evac(og2, Sf, ib, i0):
                    def go():
                        nc.sync.dma_start(out=den_d[ib], in_=Sf)
                        ot = opool.tile([128, HT, IBS], DT, tag="ot",
                                        name=f"ot{i0}")
                        for g in range(HT // 2):
                            # one double-width copy evacuates a 2-bank tile
                            if g % 2 == 0:
                                nc.vector.tensor_copy(
                                    out=ot[:, 2 * g:2 * g + 2, :], in_=og2[g])
                            else:
                                nc.scalar.activation(
                                    out=ot[:, 2 * g:2 * g + 2, :], in_=og2[g],
                                    func=mybir.ActivationFunctionType.Copy)
                        nc.scalar.dma_start(out=outT_d[:, :, i0:i0 + IBS],
                                            in_=ot)
                    pending.append(go)

                for ib in range(IB):
                    i0 = ib * IBS
                    og2 = [ps_o.tile([128, 2 * IBS], F32, tag="o",
                                     name=f"o{ib}_{g}")
                           for g in range(HT // 2)]
                    og = [og2[g // 2][:, (g % 2) * IBS:(g % 2 + 1) * IBS]
                          for g in range(HT)]
                    Sf = spool.tile([128, IBS], F32, tag="Sf", name=f"Sf{ib}")
                    for jt in range(JT):
                        sps = ps_s.tile([128, IBS], F32, tag="s")
                        # hold the previous i-block's PV/evacuation flush for
                        # two j-tiles so this block's first exps enqueue on
                        # ScalarE ahead of the evacuation copies
                        hold_flush = (jt < 2 and ib > 0)
                        if USE_DR:
                            for g in range(HT // 2):
                                nc.tensor.matmul(
                                    sps,
                                    kT[:, 2 * g:2 * g + 2,
                                       jt * 128:(jt + 1) * 128],
                                    qT[:, 2 * g:2 * g + 2, i0:i0 + IBS],
                                    start=(g == 0), stop=(g == HT // 2 - 1),
                                    perf_mode=mybir.MatmulPerfMode.DoubleRow,
                                )
                        else:
                            for ht in range(HT):
                                nc.tensor.matmul(
                                    sps,
                                    kT[:, ht, jt * 128:(jt + 1) * 128],
                                    qT[:, ht, i0:i0 + IBS],
                                    start=(ht == 0), stop=(ht == HT - 1),
                                )
                        if not hold_flush:
                            flush()
                        # p-1 shift: store fp8(exp(s)-1) -- ~3.6x finer fp8
                        # quantization since exp args are ~N(0,0.33). The
                        # host adds back the exact +colsum(v8) rank-1 term
                        # and +NK to den. The subtract runs on the otherwise
                        # idle GpSimd engine.
                        pt = tpool.tile([128, IBS], DT, tag="pt")
                        nc.scalar.activation(
                            out=pt, in_=sps,
                            func=mybir.ActivationFunctionType.Exp,
                            scale=SCALE / M_SCALE,
                            bias=dbs[:, jt:jt + 1],
                        )
                        if jt % 2 == 0:
                            nc.vector.tensor_scalar_sub(p_ap(jt), pt, 1.0)
                        else:
                            nc.scalar.activation(
                                out=p_ap(jt), in_=pt,
                                func=mybir.ActivationFunctionType.Identity,
                                bias=neg1,
                            )
                        if jt == 0:
                            nc.vector.tensor_copy(out=Sf, in_=p_ap(jt))
                        else:
                            nc.vector.tensor_add(Sf, Sf, p_ap(jt))
                        # pair 0 is deferred one extra j-tile so the previous
                        # i-block's PSUM evacuation (DVE/ScalarE) finishes
                        # before its banks are re-accumulated
                        if jt == 2 and NF8 >= 2:
                            emit_pv8(og, 0)
                        if jt < NF8:
                            if (jt % 2 == 1 and jt > 1
                                    and not (ib == IB - 1 and jt == JT - 1)):
                                emit_pv8(og, jt // 2)
                        elif not (ib == IB - 1 and jt == JT - 1):
                            emit_pv(og, jt, i0)
                    if ib < IB - 1:
                        emit_den_and_evac(og2, Sf, ib, i0)
                    else:
                        # eager epilogue: interleave the final j-tile pair's
                        # PV matmuls with per-SINGLE-h-tile evacuation and
                        # DMA so the output drains as early as possible.
                        def epilogue(og=og, og2=og2, Sf=Sf, ib=ib, i0=i0):
                            nc.sync.dma_start(out=den_d[ib], in_=Sf)
                            ot = opool.tile([128, HT, IBS], DT, tag="ot",
                                            name=f"ot{i0}")
                            for ht in range(HT):
                                if NF8 == JT:
                                    nc.tensor.matmul(
                                        og[ht],
                                        v8p[JT // 2 - 1][:, :,
                                                         ht * 128:(ht + 1) * 128],
                                        p8_t[JT // 2 - 1],
                                        start=False, stop=True,
                                        perf_mode=mybir.MatmulPerfMode.DoubleRow,
                                    )
                                else:
                                    nc.tensor.matmul(
                                        og[ht],
                                        vv[JT - 1][:, ht * 128:(ht + 1) * 128],
                                        p_t[JT - 1],
                                        start=False, stop=True,
                                    )
                                if ht % 2 == 1:
                                    g = ht // 2
                                    if g % 2 == 0:
                                        nc.vector.tensor_copy(
                                            out=ot[:, ht - 1:ht + 1, :],
                                            in_=og2[g])
                                    else:
                                        nc.scalar.activation(
                                            out=ot[:, ht - 1:ht + 1, :],
                                            in_=og2[g],
                                            func=mybir.ActivationFunctionType.Copy)
                                    dma = (nc.sync.dma_start if ht % 4 == 1
                                           else nc.scalar.dma_start)
                                    dma(out=outT_d[:, ht - 1:ht + 1,
                                                   i0:i0 + IBS],
                                        in_=ot[:, ht - 1:ht + 1, :])
                        pending.append(epilogue)
                flush()
    nc.compile()
    return nc


@lru_cache(maxsize=1)
def _cached_program():
    return build_program()


def _prep_in_maps(x, W_qkv, b_qkv):
    import ml_dtypes

    x = np.asarray(x, dtype=np.float32)
    W_qkv = np.asarray(W_qkv, dtype=np.float32)
    b_qkv = np.asarray(b_qkv, dtype=np.float32)
    Wq, Wk, Wv = W_qkv[:, 0:H], W_qkv[:, H:2 * H], W_qkv[:, 2 * H:3 * H]
    bq = b_qkv[0:H]
    # scores_raw = x (Wq Wk^T) x^T + per-key bias d; per-query terms cancel
    M = (M_SCALE * (Wq @ Wk.T)).astype(np.float32)           # [C, C]
    w3 = Wk @ bq                                             # [C]
    # interleave [C, 2H] -> [128, CT, 2H] so W sections are single DMAs
    w16 = np.ascontiguousarray(
        np.concatenate([M, Wv], axis=1)
        .astype(np.float16).reshape(CT, 128, 2 * H).transpose(1, 0, 2))
    bv = b_qkv[2 * H:3 * H].astype(np.float32)
    Wv16 = Wv.astype(np.float16).astype(np.float32)

    in_maps = []
    hcorrs = []
    for core in range(NCORES):
        b, kh = core // 2, core % 2
        # this core's rows: keys == own query half
        xb = x[b][kh * NK:(kh + 1) * NK]   # [NK, C] f32
        xTf = xb.T.reshape(CT, 128, NK).transpose(1, 0, 2)
        xT = np.ascontiguousarray(xTf.astype(np.float16))
        x8 = np.ascontiguousarray(xTf.astype(ml_dtypes.float8_e4m3fn))
        db = np.ascontiguousarray(
            (SCALE * (xb @ w3)).astype(np.float32).reshape(JT, 128).T)
        in_maps.append({"xT": xT, "x8": x8, "w": w16, "db": db})
        # replicate the device's v (fp16 x, fp16 Wv, fp32 accum, no bias) and
        # its fp8 quantization; vres = column-sum of the residual over the
        # fp8-stored keys. Host adds out += outer(bv + vres/NK, den) and the
        # p-1 shift's exact +colsum(v8) term.
        vdev = xb.astype(np.float16).astype(np.float32) @ Wv16
        v8 = vdev[0:NF8 * 128].astype(ml_dtypes.float8_e4m3fn).astype(np.float32)
        vres = (vdev[0:NF8 * 128] - v8).sum(axis=0)
        vsum = v8.sum(axis=0) + vdev[NF8 * 128:].astype(
            np.float16).astype(np.float32).sum(axis=0)
        hcorrs.append((bv + vres / NK, vsum))
    return in_maps, hcorrs


def _unT(o):
    # [128, HT, N] fp16 -> [H, N] fp32
    return o.astype(np.float32).transpose(1, 0, 2).reshape(H, N)


def _combine(results, hcorrs):
    out = np.empty((B, N, C), dtype=np.float32)
    for b in range(B):
        r0, r1 = results[2 * b], results[2 * b + 1]
        o0 = _unT(r0["outT"])                    # [H, N]
        d0 = r0["den"].astype(np.float32).sum(axis=1).reshape(N) + NK
        o1 = _unT(r1["outT"])
        d1 = r1["den"].astype(np.float32).sum(axis=1).reshape(N) + NK
        # rank-1 corrections: the p-1 shift's +colsum(v8) term, the v bias,
        # and the fp8-v quantization residual mean. (Both cores' outputs are
        # already in unrotated query order thanks to the AllGather placement.)
        hc0, vs0 = hcorrs[2 * b]
        hc1, vs1 = hcorrs[2 * b + 1]
        o0 = o0 + vs0[:, None] + np.outer(hc0, d0)
        o1 = o1 + vs1[:, None] + np.outer(hc1, d1)
        out[b] = ((o0 + o1) / (d0 + d1)).T
    return out


def kernel(x, W_qkv, b_qkv):
    nc = _cached_program()
    in_maps, hcorrs = _prep_in_maps(x, W_qkv, b_qkv)
    res = run_bass_kernel_spmd(nc, in_maps, core_ids=list(range(NCORES)))
    return _combine(res.results, hcorrs)

